# revision 41
# baseline (speedup 1.0000x reference)
"""AttnBlock (GroupNorm -> single-head self-attention -> residual) on 8 TRN2 cores.

Sharding: B=4 batch elements x 2 query-token halves = 8 cores (SPMD, no
collectives).  Each core receives the full (rolled) channel-major batch
element x^T [C=256, HW=4096], computes GroupNorm + k/v for all 4096
tokens, and q/scores/attention for its 2048-token half.

Structure (v7):
  * All weights/constants ship in ONE [P, 3136]-byte blob (3KB lines --
    avoids the sub-2KB DMA-line tax that serialized the x load).
  * Wo folded into Wv on the host (Wt = Wo Wv * 2^-0.5, bf16); the GN
    affine folds into Wt on-device (waT = a * Wt per contraction row,
    W b constant -> residual bias), so no bf16 hs is materialized and
    the attn@v matmul directly yields the out-projection.
  * q/k projections: fp8 DoubleRow from an fp8 hs (K=256/instruction).
  * exp split between ACT (true exp -> fp8) and DVE (Schraudolph:
    sat_u8(round(S*A8+B8)) = fp8e4m3 approx of exp(S/16-2)).
  * GroupNorm: DVE bn_stats (half 0) + ACT accum sums (half 1), one
    merged [P,2]-wide alpha/beta chain, rsqrt via bit-trick + 1 Newton.
  * k blocks 1-7, all v groups and later q blocks are woven into strip
    0's score stream; PSUM evacuations alternate ACT/DVE.
  * Tail per strip: Z row -> 1/Z -> gpsimd partition_broadcast ->
    normalize -> residual add (DVE) -> store.  The last strip uses a
    PE row-broadcast instead (shorter latency, all engines idle).

Numerics (budget 2e-2; measured ~6.6e-3): see the numpy sim in the
session notes -- fp8 q/k/hs, fp8 es (both exp paths have the same ~6%
per-element envelope as exp->fp8e4m3 rounding), bf16 fused v path.
"""

import numpy as np
import ml_dtypes

import concourse.bass as bass
import concourse.tile as tile
from concourse import bacc, mybir
from concourse.bass_utils import run_bass_kernel_spmd

dt = mybir.dt
F32, F32R, BF16, FP8 = dt.float32, dt.float32r, dt.bfloat16, dt.float8e4
U8 = dt.uint8
AF = mybir.ActivationFunctionType
ALU = mybir.AluOpType
DR = mybir.MatmulPerfMode.DoubleRow

P = 128          # partitions
C = 256          # channels
N = 4096         # tokens per batch element (64*64)
NQ = 2048        # query tokens per core
SW = 512         # query strip width
NS = NQ // SW    # 4 strips
MT = N // P      # 32 key m-tiles
MP = MT // 2     # 16 key m-tile pairs (DoubleRow)
GS = 8           # channels per group (256 / 32 groups)
EPS = 1e-6
ISCALE = 1.0 / 16.0       # attention scale c**-0.5
EBIAS = -2.0              # exp range shift; cancels in softmax
RS2 = float(2.0 ** -0.5)  # output residual scale
LN2 = float(np.log(2.0))
A8 = 8.0 * ISCALE / LN2            # schraudolph scale
B8 = 56.0 + 8.0 * EBIAS / LN2 - 0.344  # schraudolph bias (centered)

# wall blob byte offsets (per partition)
O_WQ, O_WK, O_WT = 0, 512, 1024
O_AMAT, O_ONESB, O_ONES8 = 2048, 2560, 3072
O_BOS, O_GNW, O_GNB = 3104, 3112, 3120
WALL = 3136

# exp blocks per strip position sent to the DVE (rest go to ACT)
S0_DVE = frozenset((3, 5, 9, 11, 13, 15))
S123_DVE = frozenset((3, 5, 7, 9, 11, 13))
LAG = 2

_prog_cache = {}


def _build_nc():
    nc = bacc.Bacc("TRN2", target_bir_lowering=False, debug=False, num_devices=8)

    xbf_d = nc.dram_tensor("xbf", [2, P, N], BF16, kind="ExternalInput").ap()
    wall_d = nc.dram_tensor("wall", [P, WALL], U8, kind="ExternalInput").ap()
    amat_d = nc.dram_tensor("amat", [P, P], F32, kind="ExternalInput").ap()
    onesb_d = nc.dram_tensor("onesb", [P, P], F32, kind="ExternalInput").ap()
    out_d = nc.dram_tensor("out", [2, P, NQ], F32, kind="ExternalOutput").ap()

    with tile.TileContext(nc) as tc:
        with (
            tc.tile_pool(name="singles", bufs=1) as singles,
            tc.tile_pool(name="xpool", bufs=1) as xpool,
            tc.tile_pool(name="hsp", bufs=1) as hsp,
            tc.tile_pool(name="qk", bufs=1) as qk,
            tc.tile_pool(name="vpool", bufs=1) as vpool,
            tc.tile_pool(name="espool", bufs=2) as espool,
            tc.tile_pool(name="xrpool", bufs=1) as xrpool,
            tc.tile_pool(name="rzpool", bufs=2) as rzpool,
            tc.tile_pool(name="t1pool", bufs=2) as t1pool,
            tc.tile_pool(name="finpool", bufs=2) as finpool,
            tc.tile_pool(name="small", bufs=2) as small,
            tc.tile_pool(name="ps", bufs=2, space="PSUM") as ps,      # 4 banks
            tc.tile_pool(name="pz", bufs=1, space="PSUM") as pz,      # 1 bank
            tc.tile_pool(name="po", bufs=3, space="PSUM") as po,      # 3 banks
        ):
            # ---- the weight blob: one wide fast DMA, then x ----
            wall = singles.tile([P, WALL], U8)
            nc.sync.dma_start(wall[:], wall_d)
            wq8 = wall[:, O_WQ:O_WQ + 512].bitcast(FP8).rearrange(
                "p (a b) -> p a b", b=C)
            wk8 = wall[:, O_WK:O_WK + 512].bitcast(FP8).rearrange(
                "p (a b) -> p a b", b=C)
            wt16 = wall[:, O_WT:O_WT + 1024].bitcast(BF16).rearrange(
                "p (a b) -> p a b", b=C)
            amat_t = singles.tile([P, P], F32R)
            nc.gpsimd.dma_start(amat_t[:], amat_d.bitcast(F32R))
            onesb_t = singles.tile([P, P], F32R)
            nc.gpsimd.dma_start(onesb_t[:], onesb_d.bitcast(F32R))
            amat = amat_t[:]
            onesb = onesb_t[:]
            ones8 = wall[:, O_ONES8:O_ONES8 + 32].bitcast(FP8).rearrange(
                "p (a b) -> p a b", b=16)
            bos = wall[:, O_BOS:O_BOS + 8].bitcast(F32)
            gnw = wall[:, O_GNW:O_GNW + 8].bitcast(F32)
            gnb = wall[:, O_GNB:O_GNB + 8].bitcast(F32)

            xb0 = xpool.tile([P, N], BF16, tag="xb0")
            xb1 = xpool.tile([P, N], BF16, tag="xb1")
            nc.sync.dma_start(xb0[:], xbf_d[0])
            nc.scalar.dma_start(xb1[:], xbf_d[1])
            xbs = (xb0, xb1)

            ebias = singles.tile([P, 1], F32)
            nc.vector.memset(ebias[:], EBIAS)
            magic = singles.tile([P, 2], dt.int32)
            nc.vector.memset(magic[:], 0x5F3759DF)

            # ---- PE warm-up: junk matmuls paced by the blob then by the
            # x chunks, keeping the HAM clock up until the real work ----
            wflat = wall[:, 0:512].bitcast(FP8)
            for i in range(8):
                warm = po.tile([P, SW], F32, tag="po", name=f"warmA{i}")
                nc.tensor.matmul(warm[:], wflat[:, 0:P], wflat,
                                 start=True, stop=True)

            # ---- GroupNorm stats: DVE bn_stats (half 0) + ACT accumulate
            # sums (half 1), both paced by x-chunk arrival ----
            hs8 = hsp.tile([P, 2, N], FP8, tag="hs8")
            st0 = small.tile([P, 8, 6], F32, tag="gnst0")
            g1 = small.tile([P, 4], F32, tag="gns1")
            for h in range(2):
                hsl = slice(h * 2048, (h + 1) * 2048)
                sq = small.tile([P, 2048], BF16, tag="sq")
                nc.scalar.activation(sq[:], xbs[1][:, hsl], AF.Identity,
                                     accum_out=g1[:, h:h + 1])
                sq2 = small.tile([P, 2048], BF16, tag="sq")
                nc.scalar.activation(sq2[:], xbs[1][:, hsl], AF.Square,
                                     accum_out=g1[:, 2 + h:3 + h])
            for h in range(4):
                hsl = slice(h * 1024, (h + 1) * 1024)
                xre = xbs[0][:, hsl].rearrange("p (s f) -> p s f", f=512)
                for sg in range(2):
                    nc.vector.bn_stats(st0[:, 2 * h + sg, :], xre[:, sg, :])
                warmH = po.tile([P, SW], F32, tag="po", name=f"wH{h}")
                nc.tensor.matmul(warmH[:], xbs[1][:, hsl][:, 0:P],
                                 xbs[1][:, hsl][:, 0:SW],
                                 start=True, stop=True)
                for t in range(2):
                    warm = po.tile([P, SW], F32, tag="po", name=f"wB{t}_{h}")
                    nc.tensor.matmul(warm[:], xbs[t][:, h * 1024:h * 1024 + P],
                                     xbs[t][:, h * 1024:h * 1024 + SW],
                                     start=True, stop=True)
                    warm2 = po.tile([P, SW], F32, tag="po", name=f"wC{t}_{h}")
                    nc.tensor.matmul(warm2[:], xbs[t][:, h * 1024:h * 1024 + P],
                                     xbs[t][:, h * 1024 + SW:(h + 1) * 1024],
                                     start=True, stop=True)

            # ---- merged two-half alpha/beta chain ([P,2]-wide ops) ----
            stats2 = small.tile([P, 4], F32R, tag="gnst2")
            mv = small.tile([P, 2], F32, tag="gnmv")
            nc.vector.bn_aggr(mv[:], st0[:])
            musq = small.tile([P, 1], F32, tag="gnmusq")
            nc.vector.tensor_mul(musq[:], mv[:, 0:1], mv[:, 0:1])
            nc.vector.tensor_copy(stats2[:, 0:1], mv[:, 0:1])
            nc.vector.tensor_add(stats2[:, 1:2], mv[:, 1:2], musq[:])
            tot = small.tile([P, 2], F32, tag="gnt")
            nc.vector.tensor_add(tot[:], g1[:, 0:4:2], g1[:, 1:4:2])
            nc.vector.tensor_scalar(stats2[:, 2:4], tot[:], 1.0 / N, None,
                                    ALU.mult)
            gp = ps.tile([P, 2, SW], F32, tag="ps", name="gnagg")
            nc.tensor.matmul(gp[:, 0, 0:4], amat, stats2[:],
                             start=True, stop=True)
            gs = small.tile([P, 4], F32, tag="gnagg2")
            nc.vector.tensor_copy(gs[:], gp[:, 0, 0:4])
            mus = gs[:, 0:4:2]
            gmusq = small.tile([P, 2], F32, tag="gnmusq2")
            nc.vector.tensor_mul(gmusq[:], mus, mus)
            gve = small.tile([P, 2], F32, tag="gnve")
            nc.vector.scalar_tensor_tensor(
                out=gve[:], in0=gs[:, 1:4:2], scalar=EPS, in1=gmusq[:],
                op0=ALU.add, op1=ALU.subtract)
            ysh = small.tile([P, 2], dt.int32, tag="gnsh")
            nc.vector.tensor_scalar(ysh[:], gve[:].bitcast(dt.int32),
                                    1, None, ALU.arith_shift_right)
            yi = small.tile([P, 2], dt.int32, tag="gnyi")
            nc.vector.tensor_tensor(yi[:], magic[:], ysh[:], ALU.subtract)
            y = yi[:].bitcast(F32)
            yy = small.tile([P, 2], F32, tag="gnyy")
            nc.vector.tensor_mul(yy[:], y, y)
            nc.vector.tensor_mul(yy[:], yy[:], gve[:])
            nc.vector.tensor_scalar(yy[:], yy[:], -0.5, 1.5, ALU.mult, ALU.add)
            yo = small.tile([P, 2], F32, tag="gnyo")
            nc.vector.tensor_mul(yo[:], y, yy[:])
            alpha2 = small.tile([P, 2], F32, tag="gnalpha")
            nc.vector.tensor_mul(alpha2[:], yo[:], gnw)
            atmp = small.tile([P, 2], F32, tag="gnatmp")
            nc.vector.tensor_mul(atmp[:], mus, alpha2[:])
            beta2 = small.tile([P, 2], F32, tag="gnbeta")
            nc.vector.tensor_tensor(beta2[:], gnb, atmp[:], ALU.subtract)

            # fp8 hs, token-chunk major so k/q projections unblock early
            for hh in range(2):
                for t in range(2):
                    nc.vector.tensor_scalar(
                        hs8[:, t, hh * 2048:(hh + 1) * 2048],
                        xbs[t][:, hh * 2048:(hh + 1) * 2048],
                        alpha2[:, t:t + 1], beta2[:, t:t + 1],
                        ALU.mult, ALU.add)
            for hh in range(4):
                warm = po.tile([P, SW], F32, tag="po", name=f"wD{hh}")
                nc.tensor.matmul(
                    warm[:], xbs[0][:, hh * SW:hh * SW + P],
                    xbs[0][:, hh * SW:(hh + 1) * SW],
                    start=True, stop=True)

            # GN affine folded into the fused v weights: Wt(a x + b) =
            # (Wt D_a) x + (Wt b); the constant joins the residual bias.
            waT = hsp.tile([P, 2, C], BF16, tag="waT")
            for t in range(2):
                nc.vector.tensor_scalar(waT[:, t, :], wt16[:, t, :],
                                        alpha2[:, t:t + 1], None, ALU.mult)
            beta16 = small.tile([P, 2], BF16, tag="beta16")
            nc.vector.tensor_copy(beta16[:], beta2[:])
            cstp = po.tile([P, SW], F32, tag="po", name="cstp")
            for ch in range(2):
                for t in range(2):
                    nc.tensor.matmul(cstp[:, ch:ch + 1],
                                     wt16[:, t, ch * P:(ch + 1) * P],
                                     beta16[:, t:t + 1],
                                     start=(t == 0), stop=(t == 1))
            bos2 = small.tile([P, 2], F32, tag="bos2")
            nc.vector.tensor_add(bos2[:], bos, cstp[:, 0:2])

            # xr = x * RS2 + (bos + Wt b) from the bf16 x already in SBUF
            # (bf16 residual: ~2e-3 extra error, no extra DMA); idle gpsimd.
            xr = xrpool.tile([P, 2, NQ], F32, tag="xr")
            for ch in range(2):
                for h in range(2):
                    sl = slice(h * 1024, (h + 1) * 1024)
                    nc.gpsimd.tensor_scalar(xr[:, ch, sl], xbs[ch][:, sl],
                                            RS2, bos2[:, ch:ch + 1],
                                            ALU.mult, ALU.add)

            # ---- projections.  q/k: fp8 DoubleRow; v: bf16 fused.  Only
            # q blk0 / k blk0 are up front; k blks 1-7 and the v groups
            # weave into strip 0, later q blocks into strips 0-2. ----
            qT = qk.tile([P, 2, NQ], FP8, tag="qT")
            kT = qk.tile([P, 2, N], FP8, tag="kT")
            v = vpool.tile([P, MT, C], FP8)

            def emit_qk(w8, dst, blk, eng):
                sp = ps.tile([P, 2, SW], F32, tag="ps")
                for ch in range(2):
                    nc.tensor.matmul(
                        sp[:, ch, :],
                        w8[:, :, ch * P:(ch + 1) * P],
                        hs8[:, :, blk * SW:(blk + 1) * SW],
                        start=True, stop=True, perf_mode=DR)
                d_ap = dst[:, 0:2, blk * SW:(blk + 1) * SW]
                s_ap = sp[:, 0:2, :].rearrange("p a b -> p (a b)")
                if eng == 0:
                    nc.scalar.activation(d_ap, s_ap, AF.Identity, scale=1.0)
                else:
                    nc.vector.tensor_copy(d_ap, s_ap)

            def emit_vgrp(g, eng):
                vp = ps.tile([P, 2, SW], F32, tag="ps", name=f"vp{g}")
                for mi in range(4):
                    m = 4 * g + mi
                    dst = vp[:, mi // 2, (mi % 2) * C:(mi % 2 + 1) * C]
                    for ko in range(2):
                        nc.tensor.matmul(dst,
                                         xbs[ko][:, m * P:(m + 1) * P],
                                         waT[:, ko, :], start=(ko == 0),
                                         stop=(ko == 1))
                d_ap = v[:, 4 * g:4 * g + 4, :].rearrange("p a b -> p (a b)")
                s_ap = vp[:, 0:2, :].rearrange("p a b -> p (a b)")
                if eng == 0:
                    nc.scalar.activation(d_ap, s_ap, AF.Identity, scale=1.0)
                else:
                    nc.vector.tensor_copy(d_ap, s_ap)

            emit_qk(wq8, qT, 0, 0)
            emit_qk(wk8, kT, 0, 1)

            # ---- attention strips (fp8 DoubleRow) ----
            zp = pz.tile([P, SW], F32, tag="pz")
            nc.vector.memset(zp[:], 0.0)

            def make_tail(s, ns_, ops_):
                st = {}
                final = (s == NS - 1)

                def stage0():  # DVE: Z out of psum + reciprocal
                    if final:
                        zsb = small.tile([P, SW], F32R, tag="zsbF",
                                         name=f"zsb{s}")
                        nc.vector.tensor_copy(zsb[:], zp[:])
                    else:
                        zsb = small.tile([1, SW], F32, tag="zsb",
                                         name=f"zsb{s}")
                        nc.vector.tensor_copy(zsb[:], zp[0:1, :])
                        rz1 = small.tile([1, SW], F32, tag="rz1",
                                         name=f"rz1{s}")
                        nc.vector.reciprocal_approx_fast(rz1[:], zsb[:])
                        st["rz1"] = rz1
                    st["zsb"] = zsb

                def stage1():  # broadcast Z (or 1/Z) to all partitions
                    if final:
                        # PE row-broadcast back over zp, then ACT evacuates
                        nc.tensor.matmul(zp[:], onesb, st["zsb"][:],
                                         start=True, stop=True)
                        zbr = rzpool.tile([P, SW], F32, tag="rzb",
                                          name=f"zbr{s}")
                        nc.scalar.activation(zbr[:], zp[:], AF.Identity,
                                             scale=1.0)
                        rzb = rzpool.tile([P, SW], F32, tag="rzb",
                                          name=f"rzb{s}")
                        nc.vector.reciprocal_approx_fast(rzb[:], zbr[:])
                    else:
                        rzb = rzpool.tile([P, SW], F32, tag="rzb",
                                          name=f"rzb{s}")
                        nc.gpsimd.partition_broadcast(rzb[:], st["rz1"][:])
                    st["rzb"] = rzb

                def stage2():  # DVE: normalize (ch0 first: frees the po
                    t1s = []   # buffer the next strip's op1 reuses)
                    for ch in range(2):
                        t1 = t1pool.tile([P, SW], F32, tag="t1",
                                         name=f"t1_{s}_{ch}")
                        nc.vector.tensor_mul(t1[:], ops_[ch][:], st["rzb"][:])
                        t1s.append(t1)
                    st["t1"] = t1s

                def stage3():  # DVE: residual add; store
                    for ch in range(2):
                        fin = finpool.tile([P, SW], F32, tag="fin")
                        nc.vector.tensor_add(fin[:], st["t1"][ch][:],
                                             xr[:, ch, ns_])
                        nc.sync.dma_start(out_d[ch, :, ns_], fin[:])

                return [stage0, stage1, stage2, stage3]

            pend = []
            drains = []
            for s in range(NS):
                ns = slice(s * SW, (s + 1) * SW)
                es = espool.tile([P, MT, SW], FP8, tag="es")
                op0 = po.tile([P, SW], F32, tag="po", name=f"op{s}_0")
                op1 = po.tile([P, SW], F32, tag="po", name=f"op{s}_1")
                ops = (op0, op1)

                def zav(jq, es_=es, ops_=ops):
                    nc.tensor.matmul(zp[0:1, :], ones8[:, :, 0:1],
                                     es_[:, 2 * jq:2 * jq + 2, :],
                                     start=(jq == 0), stop=(jq == MP - 1),
                                     perf_mode=DR)
                    for ch in range(2):
                        nc.tensor.matmul(
                            ops_[ch],
                            v[:, 2 * jq:2 * jq + 2, ch * P:(ch + 1) * P],
                            es_[:, 2 * jq:2 * jq + 2, :],
                            start=(jq == 0), stop=(jq == MP - 1),
                            perf_mode=DR)

                tail_at = {0: 0, 1: 1, 2: 1, 3: 3}
                dve_set = S0_DVE if s == 0 else S123_DVE
                for jp in range(MP):
                    if jp == 0:
                        for d in drains:
                            d()
                    if pend:
                        for k, at in tail_at.items():
                            if jp == at:
                                pend[k]()
                    if s == 0 and jp < 7:
                        emit_qk(wk8, kT, jp + 1, jp % 2)
                    sp = ps.tile([P, 2, SW], F32, tag="ps")
                    for i in range(2):
                        m = 2 * jp + i
                        nc.tensor.matmul(
                            sp[:, i, :],
                            kT[:, :, m * P:(m + 1) * P],
                            qT[:, :, ns],
                            start=True, stop=True, perf_mode=DR)
                    flat_es = es[:, 2 * jp:2 * jp + 2, :].rearrange(
                        "p a b -> p (a b)")
                    flat_sp = sp[:, 0:2, :].rearrange("p a b -> p (a b)")
                    if jp in dve_set:
                        nc.vector.tensor_scalar(flat_es.bitcast(U8), flat_sp,
                                                A8, B8, ALU.mult, ALU.add)
                    else:
                        nc.scalar.activation(flat_es, flat_sp, AF.Exp,
                                             bias=ebias[:], scale=ISCALE)
                    if s == 0 and jp < 8:
                        emit_vgrp(jp, (jp + 1) % 2)
                    if s < NS - 1 and jp == 4:
                        emit_qk(wq8, qT, s + 1, 0)
                    if jp >= LAG:
                        zav(jp - LAG)
                drains = [
                    (lambda jq=jq, z=zav: z(jq))
                    for jq in range(MP - LAG, MP)
                ]
                pend = make_tail(s, ns, ops)
            for dr in drains:
                dr()
            for stage in pend:
                stage()

    nc.finalize()
    return nc


def _get_nc():
    if "nc" not in _prog_cache:
        _prog_cache["nc"] = _build_nc()
    return _prog_cache["nc"]


def _make_in_maps(x, gn_weight, gn_bias, Wq, bq, Wk, bk, Wv, bv, Wo, bo):
    x = np.asarray(x, dtype=np.float32)
    f32 = lambda a: np.ascontiguousarray(np.asarray(a, dtype=np.float32))
    BF = ml_dtypes.bfloat16
    F8 = ml_dtypes.float8_e4m3fn

    def packT(b_vec):  # [256] -> [128, 2] (c_out_in, c_out_half)
        return np.ascontiguousarray(f32(b_vec).reshape(2, P).T)

    def w8(W):  # [C, C] -> [128, 2, C] fp8 of W.T
        return np.ascontiguousarray(
            np.asarray(W, np.float32).T.reshape(2, P, C).transpose(1, 0, 2)
            .astype(F8))

    Wt = (np.asarray(Wo, np.float32) @ np.asarray(Wv, np.float32)) * RS2
    wt16 = np.ascontiguousarray(
        Wt.T.reshape(2, P, C).transpose(1, 0, 2).astype(BF))

    amat = np.zeros((P, P), np.float32)
    for g in range(P // GS):
        amat[g * GS:(g + 1) * GS, g * GS:(g + 1) * GS] = 1.0 / GS
    onesb = np.zeros((P, P), np.float32)
    onesb[0, :] = 1.0

    wall = np.zeros((P, WALL), np.uint8)

    def put(off, arr):
        b = np.ascontiguousarray(arr).view(np.uint8).reshape(P, -1)
        wall[:, off:off + b.shape[1]] = b

    put(O_WQ, w8(Wq))
    put(O_WK, w8(Wk))
    put(O_WT, wt16)
    put(O_ONES8, np.ones((P, 32), F8))
    put(O_BOS, packT((np.asarray(bo, np.float32)
                      + np.asarray(Wo, np.float32) @ f32(bv)) * RS2))
    put(O_GNW, packT(gn_weight))
    put(O_GNB, packT(gn_bias))

    in_maps = []
    for core in range(8):
        b, half = core // 2, core % 2
        xt = x[b].reshape(C, N)
        if half:
            xt = np.roll(xt, -NQ, axis=1)
        xt = np.ascontiguousarray(xt).reshape(2, P, N)
        in_maps.append({
            "xbf": xt.astype(BF),
            "wall": wall,
            "amat": amat,
            "onesb": onesb,
        })
    return in_maps


def _assemble(results, B):
    out = np.empty((B, C, N), np.float32)
    for core in range(2 * B):
        b, half = core // 2, core % 2
        out[b, :, half * NQ:(half + 1) * NQ] = results[core]["out"].reshape(C, NQ)
    return out.reshape(B, C, 64, 64)


def kernel(x, gn_weight, gn_bias, Wq, bq, Wk, bk, Wv, bv, Wo, bo):
    x = np.asarray(x, dtype=np.float32)
    in_maps = _make_in_maps(x, gn_weight, gn_bias, Wq, bq, Wk, bk, Wv, bv, Wo, bo)
    nc = _get_nc()
    res = run_bass_kernel_spmd(nc, in_maps, list(range(8)))
    return _assemble(res.results, x.shape[0])


# revision 42
# speedup vs baseline: 1.0033x; 1.0033x over previous
"""AttnBlock (GroupNorm -> single-head self-attention -> residual) on 8 TRN2 cores.

Sharding: B=4 batch elements x 2 query-token halves = 8 cores (SPMD, no
collectives).  Each core receives the full (rolled) channel-major batch
element x^T [C=256, HW=4096], computes GroupNorm + k/v for all 4096
tokens, and q/scores/attention for its 2048-token half.

Structure (v7):
  * All weights/constants ship in ONE [P, 3136]-byte blob (3KB lines --
    avoids the sub-2KB DMA-line tax that serialized the x load).
  * Wo folded into Wv on the host (Wt = Wo Wv * 2^-0.5, bf16); the GN
    affine folds into Wt on-device (waT = a * Wt per contraction row,
    W b constant -> residual bias), so no bf16 hs is materialized and
    the attn@v matmul directly yields the out-projection.
  * q/k projections: fp8 DoubleRow from an fp8 hs (K=256/instruction).
  * exp split between ACT (true exp -> fp8) and DVE (Schraudolph:
    sat_u8(round(S*A8+B8)) = fp8e4m3 approx of exp(S/16-2)).
  * GroupNorm: DVE bn_stats (half 0) + ACT accum sums (half 1), one
    merged [P,2]-wide alpha/beta chain, rsqrt via bit-trick + 1 Newton.
  * k blocks 1-7, all v groups and later q blocks are woven into strip
    0's score stream; PSUM evacuations alternate ACT/DVE.
  * Tail per strip: Z row -> 1/Z -> gpsimd partition_broadcast ->
    normalize -> residual add (DVE) -> store.  The last strip uses a
    PE row-broadcast instead (shorter latency, all engines idle).

Numerics (budget 2e-2; measured ~6.6e-3): see the numpy sim in the
session notes -- fp8 q/k/hs, fp8 es (both exp paths have the same ~6%
per-element envelope as exp->fp8e4m3 rounding), bf16 fused v path.
"""

import numpy as np
import ml_dtypes

import concourse.bass as bass
import concourse.tile as tile
from concourse import bacc, mybir
from concourse.bass_utils import run_bass_kernel_spmd

dt = mybir.dt
F32, F32R, BF16, FP8 = dt.float32, dt.float32r, dt.bfloat16, dt.float8e4
U8 = dt.uint8
AF = mybir.ActivationFunctionType
ALU = mybir.AluOpType
DR = mybir.MatmulPerfMode.DoubleRow

P = 128          # partitions
C = 256          # channels
N = 4096         # tokens per batch element (64*64)
NQ = 2048        # query tokens per core
SW = 512         # query strip width
NS = NQ // SW    # 4 strips
MT = N // P      # 32 key m-tiles
MP = MT // 2     # 16 key m-tile pairs (DoubleRow)
GS = 8           # channels per group (256 / 32 groups)
EPS = 1e-6
ISCALE = 1.0 / 16.0       # attention scale c**-0.5
EBIAS = -2.0              # exp range shift; cancels in softmax
RS2 = float(2.0 ** -0.5)  # output residual scale
LN2 = float(np.log(2.0))
A8 = 8.0 * ISCALE / LN2            # schraudolph scale
B8 = 56.0 + 8.0 * EBIAS / LN2 - 0.344  # schraudolph bias (centered)

# wall blob byte offsets (per partition)
O_WQ, O_WK, O_WT = 0, 512, 1024
O_AMAT, O_ONESB, O_ONES8 = 2048, 2560, 3072
O_BOS, O_GNW, O_GNB = 3104, 3112, 3120
WALL = 3136

# exp blocks per strip position sent to the DVE (rest go to ACT)
S0_DVE = frozenset((3, 5, 9, 11, 13, 15))
S123_DVE = frozenset((3, 5, 7, 9, 11, 13))
LAG = 2

_prog_cache = {}


def _build_nc():
    nc = bacc.Bacc("TRN2", target_bir_lowering=False, debug=False, num_devices=8)

    xbf_d = nc.dram_tensor("xbf", [2, P, N], BF16, kind="ExternalInput").ap()
    wall_d = nc.dram_tensor("wall", [P, WALL], U8, kind="ExternalInput").ap()
    amat_d = nc.dram_tensor("amat", [P, P], F32, kind="ExternalInput").ap()
    onesb_d = nc.dram_tensor("onesb", [P, P], F32, kind="ExternalInput").ap()
    out_d = nc.dram_tensor("out", [2, P, NQ], F32, kind="ExternalOutput").ap()

    with tile.TileContext(nc) as tc:
        with (
            tc.tile_pool(name="singles", bufs=1) as singles,
            tc.tile_pool(name="xpool", bufs=1) as xpool,
            tc.tile_pool(name="hsp", bufs=1) as hsp,
            tc.tile_pool(name="qk", bufs=1) as qk,
            tc.tile_pool(name="vpool", bufs=1) as vpool,
            tc.tile_pool(name="espool", bufs=2) as espool,
            tc.tile_pool(name="xrpool", bufs=1) as xrpool,
            tc.tile_pool(name="rzpool", bufs=2) as rzpool,
            tc.tile_pool(name="t1pool", bufs=2) as t1pool,
            tc.tile_pool(name="finpool", bufs=2) as finpool,
            tc.tile_pool(name="small", bufs=2) as small,
            tc.tile_pool(name="ps", bufs=2, space="PSUM") as ps,      # 4 banks
            tc.tile_pool(name="pz", bufs=1, space="PSUM") as pz,      # 1 bank
            tc.tile_pool(name="po", bufs=3, space="PSUM") as po,      # 3 banks
        ):
            # ---- the weight blob: one wide fast DMA, then x ----
            wall = singles.tile([P, WALL], U8)
            nc.sync.dma_start(wall[:], wall_d)
            wq8 = wall[:, O_WQ:O_WQ + 512].bitcast(FP8).rearrange(
                "p (a b) -> p a b", b=C)
            wk8 = wall[:, O_WK:O_WK + 512].bitcast(FP8).rearrange(
                "p (a b) -> p a b", b=C)
            wt16 = wall[:, O_WT:O_WT + 1024].bitcast(BF16).rearrange(
                "p (a b) -> p a b", b=C)
            amat_t = singles.tile([P, P], F32R)
            nc.gpsimd.dma_start(amat_t[:], amat_d.bitcast(F32R))
            onesb_t = singles.tile([P, P], F32R)
            nc.gpsimd.dma_start(onesb_t[:], onesb_d.bitcast(F32R))
            amat = amat_t[:]
            onesb = onesb_t[:]
            ones8 = wall[:, O_ONES8:O_ONES8 + 32].bitcast(FP8).rearrange(
                "p (a b) -> p a b", b=16)
            bos = wall[:, O_BOS:O_BOS + 8].bitcast(F32)
            gnw = wall[:, O_GNW:O_GNW + 8].bitcast(F32)
            gnb = wall[:, O_GNB:O_GNB + 8].bitcast(F32)

            xb0 = xpool.tile([P, N], BF16, tag="xb0")
            xb1 = xpool.tile([P, N], BF16, tag="xb1")
            for h in range(2):
                sl = slice(h * 2048, (h + 1) * 2048)
                nc.sync.dma_start(xb0[:, sl], xbf_d[0, :, sl])
                nc.scalar.dma_start(xb1[:, sl], xbf_d[1, :, sl])
            xbs = (xb0, xb1)

            ebias = singles.tile([P, 1], F32)
            nc.vector.memset(ebias[:], EBIAS)
            magic = singles.tile([P, 2], dt.int32)
            nc.vector.memset(magic[:], 0x5F3759DF)

            # ---- PE warm-up: junk matmuls paced by the blob then by the
            # x chunks, keeping the HAM clock up until the real work ----
            wflat = wall[:, 0:512].bitcast(FP8)
            for i in range(8):
                warm = po.tile([P, SW], F32, tag="po", name=f"warmA{i}")
                nc.tensor.matmul(warm[:], wflat[:, 0:P], wflat,
                                 start=True, stop=True)

            # ---- GroupNorm stats: DVE bn_stats (half 0) + ACT accumulate
            # sums (half 1), both paced by x-chunk arrival ----
            hs8 = hsp.tile([P, 2, N], FP8, tag="hs8")
            st0 = small.tile([P, 8, 6], F32, tag="gnst0")
            g1 = small.tile([P, 4], F32, tag="gns1")
            for h in range(2):
                hsl = slice(h * 2048, (h + 1) * 2048)
                sq = small.tile([P, 2048], BF16, tag="sq")
                nc.scalar.activation(sq[:], xbs[1][:, hsl], AF.Identity,
                                     accum_out=g1[:, h:h + 1])
                sq2 = small.tile([P, 2048], BF16, tag="sq")
                nc.scalar.activation(sq2[:], xbs[1][:, hsl], AF.Square,
                                     accum_out=g1[:, 2 + h:3 + h])
            for h in range(4):
                hsl = slice(h * 1024, (h + 1) * 1024)
                xre = xbs[0][:, hsl].rearrange("p (s f) -> p s f", f=512)
                for sg in range(2):
                    nc.vector.bn_stats(st0[:, 2 * h + sg, :], xre[:, sg, :])
                warmH = po.tile([P, SW], F32, tag="po", name=f"wH{h}")
                nc.tensor.matmul(warmH[:], xbs[1][:, hsl][:, 0:P],
                                 xbs[1][:, hsl][:, 0:SW],
                                 start=True, stop=True)
                for t in range(2):
                    warm = po.tile([P, SW], F32, tag="po", name=f"wB{t}_{h}")
                    nc.tensor.matmul(warm[:], xbs[t][:, h * 1024:h * 1024 + P],
                                     xbs[t][:, h * 1024:h * 1024 + SW],
                                     start=True, stop=True)
                    warm2 = po.tile([P, SW], F32, tag="po", name=f"wC{t}_{h}")
                    nc.tensor.matmul(warm2[:], xbs[t][:, h * 1024:h * 1024 + P],
                                     xbs[t][:, h * 1024 + SW:(h + 1) * 1024],
                                     start=True, stop=True)

            # ---- merged two-half alpha/beta chain ([P,2]-wide ops) ----
            stats2 = small.tile([P, 4], F32R, tag="gnst2")
            mv = small.tile([P, 2], F32, tag="gnmv")
            nc.vector.bn_aggr(mv[:], st0[:])
            musq = small.tile([P, 1], F32, tag="gnmusq")
            nc.vector.tensor_mul(musq[:], mv[:, 0:1], mv[:, 0:1])
            nc.vector.tensor_copy(stats2[:, 0:1], mv[:, 0:1])
            nc.vector.tensor_add(stats2[:, 1:2], mv[:, 1:2], musq[:])
            tot = small.tile([P, 2], F32, tag="gnt")
            nc.vector.tensor_add(tot[:], g1[:, 0:4:2], g1[:, 1:4:2])
            nc.vector.tensor_scalar(stats2[:, 2:4], tot[:], 1.0 / N, None,
                                    ALU.mult)
            gp = ps.tile([P, 2, SW], F32, tag="ps", name="gnagg")
            nc.tensor.matmul(gp[:, 0, 0:4], amat, stats2[:],
                             start=True, stop=True)
            gs = small.tile([P, 4], F32, tag="gnagg2")
            nc.vector.tensor_copy(gs[:], gp[:, 0, 0:4])
            mus = gs[:, 0:4:2]
            gmusq = small.tile([P, 2], F32, tag="gnmusq2")
            nc.vector.tensor_mul(gmusq[:], mus, mus)
            gve = small.tile([P, 2], F32, tag="gnve")
            nc.vector.scalar_tensor_tensor(
                out=gve[:], in0=gs[:, 1:4:2], scalar=EPS, in1=gmusq[:],
                op0=ALU.add, op1=ALU.subtract)
            ysh = small.tile([P, 2], dt.int32, tag="gnsh")
            nc.vector.tensor_scalar(ysh[:], gve[:].bitcast(dt.int32),
                                    1, None, ALU.arith_shift_right)
            yi = small.tile([P, 2], dt.int32, tag="gnyi")
            nc.vector.tensor_tensor(yi[:], magic[:], ysh[:], ALU.subtract)
            y = yi[:].bitcast(F32)
            yy = small.tile([P, 2], F32, tag="gnyy")
            nc.vector.tensor_mul(yy[:], y, y)
            nc.vector.tensor_mul(yy[:], yy[:], gve[:])
            nc.vector.tensor_scalar(yy[:], yy[:], -0.5, 1.5, ALU.mult, ALU.add)
            yo = small.tile([P, 2], F32, tag="gnyo")
            nc.vector.tensor_mul(yo[:], y, yy[:])
            alpha2 = small.tile([P, 2], F32, tag="gnalpha")
            nc.vector.tensor_mul(alpha2[:], yo[:], gnw)
            atmp = small.tile([P, 2], F32, tag="gnatmp")
            nc.vector.tensor_mul(atmp[:], mus, alpha2[:])
            beta2 = small.tile([P, 2], F32, tag="gnbeta")
            nc.vector.tensor_tensor(beta2[:], gnb, atmp[:], ALU.subtract)

            # fp8 hs, token-chunk major so k/q projections unblock early
            for hh in range(2):
                for t in range(2):
                    nc.vector.tensor_scalar(
                        hs8[:, t, hh * 2048:(hh + 1) * 2048],
                        xbs[t][:, hh * 2048:(hh + 1) * 2048],
                        alpha2[:, t:t + 1], beta2[:, t:t + 1],
                        ALU.mult, ALU.add)
            for hh in range(4):
                warm = po.tile([P, SW], F32, tag="po", name=f"wD{hh}")
                nc.tensor.matmul(
                    warm[:], xbs[0][:, hh * SW:hh * SW + P],
                    xbs[0][:, hh * SW:(hh + 1) * SW],
                    start=True, stop=True)

            # GN affine folded into the fused v weights: Wt(a x + b) =
            # (Wt D_a) x + (Wt b); the constant joins the residual bias.
            waT = hsp.tile([P, 2, C], BF16, tag="waT")
            for t in range(2):
                nc.vector.tensor_scalar(waT[:, t, :], wt16[:, t, :],
                                        alpha2[:, t:t + 1], None, ALU.mult)
            beta16 = small.tile([P, 2], BF16, tag="beta16")
            nc.vector.tensor_copy(beta16[:], beta2[:])
            cstp = po.tile([P, SW], F32, tag="po", name="cstp")
            for ch in range(2):
                for t in range(2):
                    nc.tensor.matmul(cstp[:, ch:ch + 1],
                                     wt16[:, t, ch * P:(ch + 1) * P],
                                     beta16[:, t:t + 1],
                                     start=(t == 0), stop=(t == 1))
            bos2 = small.tile([P, 2], F32, tag="bos2")
            nc.vector.tensor_add(bos2[:], bos, cstp[:, 0:2])

            # xr = x * RS2 + (bos + Wt b) from the bf16 x already in SBUF
            # (bf16 residual: ~2e-3 extra error, no extra DMA); idle gpsimd.
            xr = xrpool.tile([P, 2, NQ], F32, tag="xr")
            for ch in range(2):
                for h in range(2):
                    sl = slice(h * 1024, (h + 1) * 1024)
                    nc.gpsimd.tensor_scalar(xr[:, ch, sl], xbs[ch][:, sl],
                                            RS2, bos2[:, ch:ch + 1],
                                            ALU.mult, ALU.add)

            # ---- projections.  q/k: fp8 DoubleRow; v: bf16 fused.  Only
            # q blk0 / k blk0 are up front; k blks 1-7 and the v groups
            # weave into strip 0, later q blocks into strips 0-2. ----
            qT = qk.tile([P, 2, NQ], FP8, tag="qT")
            kT = qk.tile([P, 2, N], FP8, tag="kT")
            v = vpool.tile([P, MT, C], FP8)

            def emit_qk(w8, dst, blk, eng):
                sp = ps.tile([P, 2, SW], F32, tag="ps")
                for ch in range(2):
                    nc.tensor.matmul(
                        sp[:, ch, :],
                        w8[:, :, ch * P:(ch + 1) * P],
                        hs8[:, :, blk * SW:(blk + 1) * SW],
                        start=True, stop=True, perf_mode=DR)
                d_ap = dst[:, 0:2, blk * SW:(blk + 1) * SW]
                s_ap = sp[:, 0:2, :].rearrange("p a b -> p (a b)")
                if eng == 0:
                    nc.scalar.activation(d_ap, s_ap, AF.Identity, scale=1.0)
                else:
                    nc.vector.tensor_copy(d_ap, s_ap)

            def emit_vgrp(g, eng):
                vp = ps.tile([P, 2, SW], F32, tag="ps", name=f"vp{g}")
                for mi in range(4):
                    m = 4 * g + mi
                    dst = vp[:, mi // 2, (mi % 2) * C:(mi % 2 + 1) * C]
                    for ko in range(2):
                        nc.tensor.matmul(dst,
                                         xbs[ko][:, m * P:(m + 1) * P],
                                         waT[:, ko, :], start=(ko == 0),
                                         stop=(ko == 1))
                d_ap = v[:, 4 * g:4 * g + 4, :].rearrange("p a b -> p (a b)")
                s_ap = vp[:, 0:2, :].rearrange("p a b -> p (a b)")
                if eng == 0:
                    nc.scalar.activation(d_ap, s_ap, AF.Identity, scale=1.0)
                else:
                    nc.vector.tensor_copy(d_ap, s_ap)

            emit_qk(wq8, qT, 0, 0)
            emit_qk(wk8, kT, 0, 1)

            # ---- attention strips (fp8 DoubleRow) ----
            zp = pz.tile([P, SW], F32, tag="pz")
            nc.vector.memset(zp[:], 0.0)

            def make_tail(s, ns_, ops_):
                st = {}
                final = (s == NS - 1)

                def stage0():  # DVE: Z out of psum + reciprocal
                    if final:
                        zsb = small.tile([P, SW], F32R, tag="zsbF",
                                         name=f"zsb{s}")
                        nc.vector.tensor_copy(zsb[:], zp[:])
                    else:
                        zsb = small.tile([1, SW], F32, tag="zsb",
                                         name=f"zsb{s}")
                        nc.vector.tensor_copy(zsb[:], zp[0:1, :])
                        rz1 = small.tile([1, SW], F32, tag="rz1",
                                         name=f"rz1{s}")
                        nc.vector.reciprocal_approx_fast(rz1[:], zsb[:])
                        st["rz1"] = rz1
                    st["zsb"] = zsb

                def stage1():  # broadcast Z (or 1/Z) to all partitions
                    if final:
                        # PE row-broadcast back over zp, then ACT evacuates
                        nc.tensor.matmul(zp[:], onesb, st["zsb"][:],
                                         start=True, stop=True)
                        zbr = rzpool.tile([P, SW], F32, tag="rzb",
                                          name=f"zbr{s}")
                        nc.scalar.activation(zbr[:], zp[:], AF.Identity,
                                             scale=1.0)
                        rzb = rzpool.tile([P, SW], F32, tag="rzb",
                                          name=f"rzb{s}")
                        nc.vector.reciprocal_approx_fast(rzb[:], zbr[:])
                    else:
                        rzb = rzpool.tile([P, SW], F32, tag="rzb",
                                          name=f"rzb{s}")
                        nc.gpsimd.partition_broadcast(rzb[:], st["rz1"][:])
                    st["rzb"] = rzb

                def stage2():  # DVE: normalize (ch0 first: frees the po
                    t1s = []   # buffer the next strip's op1 reuses)
                    for ch in range(2):
                        t1 = t1pool.tile([P, SW], F32, tag="t1",
                                         name=f"t1_{s}_{ch}")
                        nc.vector.tensor_mul(t1[:], ops_[ch][:], st["rzb"][:])
                        t1s.append(t1)
                    st["t1"] = t1s

                def stage3():  # DVE: residual add; store
                    for ch in range(2):
                        fin = finpool.tile([P, SW], F32, tag="fin")
                        nc.vector.tensor_add(fin[:], st["t1"][ch][:],
                                             xr[:, ch, ns_])
                        nc.sync.dma_start(out_d[ch, :, ns_], fin[:])

                return [stage0, stage1, stage2, stage3]

            pend = []
            drains = []
            for s in range(NS):
                ns = slice(s * SW, (s + 1) * SW)
                es = espool.tile([P, MT, SW], FP8, tag="es")
                op0 = po.tile([P, SW], F32, tag="po", name=f"op{s}_0")
                op1 = po.tile([P, SW], F32, tag="po", name=f"op{s}_1")
                ops = (op0, op1)

                def zav(jq, es_=es, ops_=ops):
                    nc.tensor.matmul(zp[0:1, :], ones8[:, :, 0:1],
                                     es_[:, 2 * jq:2 * jq + 2, :],
                                     start=(jq == 0), stop=(jq == MP - 1),
                                     perf_mode=DR)
                    for ch in range(2):
                        nc.tensor.matmul(
                            ops_[ch],
                            v[:, 2 * jq:2 * jq + 2, ch * P:(ch + 1) * P],
                            es_[:, 2 * jq:2 * jq + 2, :],
                            start=(jq == 0), stop=(jq == MP - 1),
                            perf_mode=DR)

                tail_at = {0: 0, 1: 1, 2: 1, 3: 3}
                dve_set = S0_DVE if s == 0 else S123_DVE
                for jp in range(MP):
                    if jp == 0:
                        for d in drains:
                            d()
                    if pend:
                        for k, at in tail_at.items():
                            if jp == at:
                                pend[k]()
                    if s == 0 and jp < 7:
                        emit_qk(wk8, kT, jp + 1, jp % 2)
                    sp = ps.tile([P, 2, SW], F32, tag="ps")
                    for i in range(2):
                        m = 2 * jp + i
                        nc.tensor.matmul(
                            sp[:, i, :],
                            kT[:, :, m * P:(m + 1) * P],
                            qT[:, :, ns],
                            start=True, stop=True, perf_mode=DR)
                    flat_es = es[:, 2 * jp:2 * jp + 2, :].rearrange(
                        "p a b -> p (a b)")
                    flat_sp = sp[:, 0:2, :].rearrange("p a b -> p (a b)")
                    if jp in dve_set:
                        nc.vector.tensor_scalar(flat_es.bitcast(U8), flat_sp,
                                                A8, B8, ALU.mult, ALU.add)
                    else:
                        nc.scalar.activation(flat_es, flat_sp, AF.Exp,
                                             bias=ebias[:], scale=ISCALE)
                    if s == 0 and jp < 8:
                        emit_vgrp(jp, (jp + 1) % 2)
                    if s < NS - 1 and jp == 4:
                        emit_qk(wq8, qT, s + 1, 0)
                    if jp >= LAG:
                        zav(jp - LAG)
                drains = [
                    (lambda jq=jq, z=zav: z(jq))
                    for jq in range(MP - LAG, MP)
                ]
                pend = make_tail(s, ns, ops)
            for dr in drains:
                dr()
            for stage in pend:
                stage()

    nc.finalize()
    return nc


def _get_nc():
    if "nc" not in _prog_cache:
        _prog_cache["nc"] = _build_nc()
    return _prog_cache["nc"]


def _make_in_maps(x, gn_weight, gn_bias, Wq, bq, Wk, bk, Wv, bv, Wo, bo):
    x = np.asarray(x, dtype=np.float32)
    f32 = lambda a: np.ascontiguousarray(np.asarray(a, dtype=np.float32))
    BF = ml_dtypes.bfloat16
    F8 = ml_dtypes.float8_e4m3fn

    def packT(b_vec):  # [256] -> [128, 2] (c_out_in, c_out_half)
        return np.ascontiguousarray(f32(b_vec).reshape(2, P).T)

    def w8(W):  # [C, C] -> [128, 2, C] fp8 of W.T
        return np.ascontiguousarray(
            np.asarray(W, np.float32).T.reshape(2, P, C).transpose(1, 0, 2)
            .astype(F8))

    Wt = (np.asarray(Wo, np.float32) @ np.asarray(Wv, np.float32)) * RS2
    wt16 = np.ascontiguousarray(
        Wt.T.reshape(2, P, C).transpose(1, 0, 2).astype(BF))

    amat = np.zeros((P, P), np.float32)
    for g in range(P // GS):
        amat[g * GS:(g + 1) * GS, g * GS:(g + 1) * GS] = 1.0 / GS
    onesb = np.zeros((P, P), np.float32)
    onesb[0, :] = 1.0

    wall = np.zeros((P, WALL), np.uint8)

    def put(off, arr):
        b = np.ascontiguousarray(arr).view(np.uint8).reshape(P, -1)
        wall[:, off:off + b.shape[1]] = b

    put(O_WQ, w8(Wq))
    put(O_WK, w8(Wk))
    put(O_WT, wt16)
    put(O_ONES8, np.ones((P, 32), F8))
    put(O_BOS, packT((np.asarray(bo, np.float32)
                      + np.asarray(Wo, np.float32) @ f32(bv)) * RS2))
    put(O_GNW, packT(gn_weight))
    put(O_GNB, packT(gn_bias))

    in_maps = []
    for core in range(8):
        b, half = core // 2, core % 2
        xt = x[b].reshape(C, N)
        if half:
            xt = np.roll(xt, -NQ, axis=1)
        xt = np.ascontiguousarray(xt).reshape(2, P, N)
        in_maps.append({
            "xbf": xt.astype(BF),
            "wall": wall,
            "amat": amat,
            "onesb": onesb,
        })
    return in_maps


def _assemble(results, B):
    out = np.empty((B, C, N), np.float32)
    for core in range(2 * B):
        b, half = core // 2, core % 2
        out[b, :, half * NQ:(half + 1) * NQ] = results[core]["out"].reshape(C, NQ)
    return out.reshape(B, C, 64, 64)


def kernel(x, gn_weight, gn_bias, Wq, bq, Wk, bk, Wv, bv, Wo, bo):
    x = np.asarray(x, dtype=np.float32)
    in_maps = _make_in_maps(x, gn_weight, gn_bias, Wq, bq, Wk, bk, Wv, bv, Wo, bo)
    nc = _get_nc()
    res = run_bass_kernel_spmd(nc, in_maps, list(range(8)))
    return _assemble(res.results, x.shape[0])
